# revision 15
# baseline (speedup 1.0000x reference)
"""BWGNN-Hetero forward on 8 Trainium2 NeuronCores.

Node-sharded (N/8 nodes per core). Per relation: two polynomial-propagation
steps; segment-sum gathers per-edge source rows (dma_gather, bf16 tables
with 256B row stride, 4 int16-indexable super-shards) and reduces them with
PE matmuls against on-device-built one-hot selection matrices into PSUM.
Node state is feature-major, bf16, and HALF-PACKED: a [128, P/2] tile holds
features of nodes [0,P/2) on partitions 0:64 and of [P/2,P) on partitions
64:128 (matmuls address the upper half via tile_position).
Halo exchange = AllGather of the scaled node table after each step.

Windows (128 dst nodes) are processed in groups of 4 sharing one 512-col
PSUM tile, so epilogues / MLP / head run as 512-wide ops (4x fewer
instructions). Bounce-table writes are batched 8 windows per DMA via a
strided access pattern.

SPMD: one program for all 8 cores; the edge layout is padded to a common
structure (per-(window,shard) chunk capacity = max over cores) so the
instruction stream is core-invariant while indices/dst data are inputs.
"""

import numpy as np
import ml_dtypes

import concourse.bass as bass
import concourse.mybir as mybir
import concourse.tile as tile
from concourse import ap_utils
from concourse.bass import MemorySpace

N_CORES = 8
H = 64
C_OUT = 2
IN_F = 128
WIN = 128
CALL_MAX = 3840          # 30 chunks/call; fine with single_packet=False
SINGLE_PACKET = False
SHARD_ROWS = 25000       # int16-indexable table super-shard
SENT = 1024.0
GW = 4                   # windows per PSUM group (4*128 = 512 psum cols)
SEL_BATCH = 32           # selection matrices built per DVE op

THETAS = np.array([[3.0, -3.0, 0.75],
                   [0.0, 3.0, -1.50],
                   [0.0, 0.0, 0.75]], dtype=np.float32)

BF16 = ml_dtypes.bfloat16
LAST_BUILD = None

# ---------------------------------------------------------------- bir fixes


def _fix_sync_waits(nc, max_waits=1):
    """This walrus build rejects >1 sync-wait per instruction; move excess
    waits onto fresh nops on the same engine queue."""
    counter = [0]
    for fn in nc.m.functions:
        for bb in fn.blocks:
            new_insts = []
            for inst in bb.instructions:
                si = inst.sync_info
                if si is None or not si.on_wait or len(si.on_wait) <= max_waits:
                    new_insts.append(inst)
                    continue
                waits = list(si.on_wait)
                for w in waits[max_waits:]:
                    counter[0] += 1
                    nop = mybir.InstNoOp(name=f"waitsplit_{counter[0]}", ins=[], outs=[])
                    nop.engine = inst.engine
                    nop.sync_info = mybir.SyncInfo(on_wait=[w], on_update=[])
                    nc.register_instruction(nop)
                    new_insts.append(nop)
                inst.sync_info = mybir.SyncInfo(
                    on_wait=waits[:max_waits], on_update=list(si.on_update))
                new_insts.append(inst)
            if len(new_insts) != len(bb.instructions):
                bb.instructions[:] = new_insts


def _insert_library_loads(nc):
    import bass_rust as _bass_rust
    from concourse.library_config import all_libraries, standard
    mask = {}
    for lib in all_libraries:
        for t in lib.instructions:
            mask[t] = mask.get(t, 0) | (1 << lib.index)
    _bass_rust.insert_library_loads(nc, mask, len(all_libraries), standard.index)


def _lower_library_reloads(nc):
    """Rewrite the pseudo library-reload into the raw 64B PSEUDO_INST struct
    this walrus can encode (not sim-executable; call only before HW runs)."""
    import bass_rust as _bass_rust
    isa = nc.isa
    PO = isa.get_enum("NEURON_ISA_TPB_PSEUDO_OPCODE")
    for fn in nc.m.functions:
        for bb in fn.blocks:
            for i, inst in enumerate(bb.instructions):
                if not isinstance(inst, _bass_rust.InstPseudoReloadLibraryIndex):
                    continue
                raw = nc.engines[inst.engine]._isa(
                    isa.Opcode.NEURON_ISA_TPB_OPCODE_PSEUDO_INST,
                    {"pseudo_opcode":
                         PO.NEURON_ISA_TPB_PSEUDO_OPCODE_PSEUDO_LIBRARY_RELOAD_INDEX.value,
                     "lib_index": inst.lib_index},
                    "NEURON_ISA_TPB_PSEUDO_LIBRARY_RELOAD_INDEX_STRUCT",
                    [], [], True)
                raw.engine = inst.engine
                raw.sync_info = inst.sync_info
                nc.register_instruction(raw, overwrite=True)
                bb.instructions[i] = raw


def _dma_gather(gp, out_ap, in_ap, idxs_ap, num_idxs, num_idxs_reg, elem_size,
                elem_step):
    """dma_gather with the elem_size%256 assert relaxed (row stride must
    still be a 256B multiple; validated on HW)."""
    assert idxs_ap.dtype == mybir.dt.int16
    assert in_ap.dtype == out_ap.dtype
    assert in_ap.space == MemorySpace.DRAM
    assert idxs_ap.space == MemorySpace.SBUF and out_ap.space == MemorySpace.SBUF
    assert ap_utils.ap_is_contiguous(out_ap.ap[1:])
    assert ap_utils.ap_is_contiguous(idxs_ap.ap[1:])
    assert in_ap.ap[-1][1] == out_ap.ap[-1][1] == elem_size
    assert out_ap.ap[0][1] * out_ap.ap[1][1] == ((num_idxs + 127) // 128) * 128
    assert in_ap.ap[0][0] == elem_step
    stride_bytes = elem_step * mybir.dt.size(in_ap.dtype)
    assert stride_bytes % 256 == 0 and stride_bytes // 256 < 256
    # single_packet concatenates each DMA lane's stream into one packet;
    # a packet is capped at 64 descriptors (hangs the SDMA engine beyond).
    assert not SINGLE_PACKET or num_idxs // 16 <= 64
    _in_ap = gp.lower_ap_dma(in_ap, for_custom_bir_dma=True)
    _idxs_ap = gp.lower_ap(idxs_ap)
    _out_ap = gp.lower_ap(out_ap)
    return gp.add_instruction(
        mybir.InstDMAGatherAnt(
            name=gp.bass.get_next_instruction_name(),
            ins=[*_in_ap, _idxs_ap, gp.lower_val_access(gp.to_reg(num_idxs_reg))],
            outs=[_out_ap],
            transpose=False, num_idxs=num_idxs, elem_size=elem_size,
            stride_bytes_256=stride_bytes // 256, gen_mode=0,
            single_packet=SINGLE_PACKET, queue_num=0, sbuf_tokens_per_rank=0,
            sbuf_free_dim_per_rank=0, sbuf_free_dim_pad_per_rank=0,
            sbuf_byte_offset=0))


# ---------------------------------------------------------------- host plan


def _wrap_idx(idx):
    """[n] -> [128, n/16] int16: idx i at [i%16, i//16], replicated for the
    8 GPSIMD cores across partition groups of 16."""
    n = len(idx)
    assert n % 16 == 0
    w = np.ascontiguousarray(idx.astype(np.int16).reshape(n // 16, 16).T)
    return np.tile(w, (8, 1))


class RelPlan:
    """Common (cross-core) structure + per-core data for one relation."""


def _plan_relation(src, dst, N, n_local):
    n_shards = (N + SHARD_ROWS - 1) // SHARD_ROWS
    n_win = (n_local + WIN - 1) // WIN

    cores = []
    counts = np.zeros((N_CORES, n_win, n_shards), np.int64)
    for c in range(N_CORES):
        lo = c * n_local
        m = (dst >= lo) & (dst < lo + n_local)
        s = src[m]
        d = dst[m] - lo
        sh = s // SHARD_ROWS
        w = d // WIN
        order = np.lexsort((d, w, sh))
        s, d, sh, w = s[order], d[order], sh[order], w[order]
        counts[c] = np.bincount(w * n_shards + sh,
                                minlength=n_win * n_shards).reshape(n_win, n_shards)
        cores.append((s - sh * SHARD_ROWS, d, sh, w))

    cap = np.ceil(counts.max(axis=0) / 128).astype(np.int64)

    chunk_ws = []
    chunk_base = np.zeros((n_win, n_shards), np.int64)
    shard_first_chunk = []
    g = 0
    for sh in range(n_shards):
        shard_first_chunk.append(g)
        for w in range(n_win):
            chunk_base[w, sh] = g
            for _ in range(cap[w, sh]):
                chunk_ws.append((w, sh))
                g += 1
    n_chunks = g
    n_slots = n_chunks * 128

    calls = []
    chunk2call = {}
    for sh in range(n_shards):
        c0 = shard_first_chunk[sh]
        c1 = shard_first_chunk[sh + 1] if sh + 1 < n_shards else n_chunks
        nch = c1 - c0
        off = 0
        while off < nch:
            take = min(CALL_MAX // 128, nch - off)
            cid = len(calls)
            calls.append(((c0 + off) * 128, take * 128, sh))
            for j in range(take):
                chunk2call[c0 + off + j] = (cid, j)
            off += take

    win_chunks = [[] for _ in range(n_win)]
    for gid, (w, sh) in enumerate(chunk_ws):
        win_chunks[w].append(gid)
    gid2ipos = np.zeros(max(1, n_chunks), np.int64)
    ipos = 0
    for w in range(n_win):
        for gid in win_chunks[w]:
            gid2ipos[gid] = ipos
            ipos += 1
    call_first_window = [min((chunk_ws[g][0] for g in
                              range(calls[cid][0] // 128,
                                    calls[cid][0] // 128 + calls[cid][1] // 128)),
                             default=0)
                         for cid in range(len(calls))]

    idx_data = np.zeros((N_CORES, n_slots), np.int64)
    dst_data = np.full((N_CORES, n_slots), -1.0, np.float64)
    for c in range(N_CORES):
        s_loc, d, sh, w = cores[c]
        pos = 0
        for shv in range(n_shards):
            for wv in range(n_win):
                cnt = counts[c, wv, shv]
                if cnt == 0:
                    continue
                b = chunk_base[wv, shv] * 128
                idx_data[c, b:b + cnt] = s_loc[pos:pos + cnt]
                dst_data[c, b:b + cnt] = d[pos:pos + cnt] - wv * WIN
                pos += cnt
        assert pos == len(s_loc)

    p = RelPlan()
    p.n_win = n_win
    p.n_chunks = n_chunks
    p.calls = calls
    p.chunk2call = chunk2call
    p.win_chunks = win_chunks
    p.call_first_window = call_first_window
    p.gid2ipos = gid2ipos
    p.idx_wrapped = [np.concatenate(
        [_wrap_idx(idx_data[c][o:o + n]) for (o, n, _s) in calls], axis=1)
        for c in range(N_CORES)]
    dl = np.where(dst_data < 0, SENT, dst_data)
    p.dstloc = []
    for c in range(N_CORES):
        byg = dl[c].reshape(n_chunks, 128).T
        byi = np.empty_like(byg)
        byi[:, gid2ipos] = byg
        p.dstloc.append(np.ascontiguousarray(byi.astype(BF16)))
    return p


# ---------------------------------------------------------------- builder


def build_nc(plans, n_local):
    P_NODES = ((n_local + 255) // 256) * 256
    HALF = P_NODES // 2
    PK = HALF // 128          # windows per partition-half
    n_win_real = (n_local + 127) // 128
    fdt = mybir.dt.float32
    bdt = mybir.dt.bfloat16
    iw_cols = [p.idx_wrapped[0].shape[1] for p in plans]
    dl_cols = [p.dstloc[0].shape[1] for p in plans]

    nc = bass.Bass(num_devices=N_CORES)

    in_featT = nc.dram_tensor("in_featT", [IN_F, P_NODES], bdt, kind="ExternalInput")
    wts = {}
    for nm, shape, dt in [
            ("W1T", [IN_F, H], bdt), ("W2T", [128, H], bdt),
            ("M0T", [128, H], bdt), ("M1T", [128, H], bdt), ("M2T", [128, H], bdt),
            ("W4T", [128, C_OUT], bdt),
            ("b1c", [128, 1], fdt), ("b2c", [128, 1], fdt),
            ("b3c", [128, 1], fdt), ("b4c", [128, 1], fdt),
            ("iota", [128, 128], bdt), ("identT", [128, H], bdt)]:
        wts[nm] = nc.dram_tensor(nm, shape, dt, kind="ExternalInput")
    dinv_d = [nc.dram_tensor(f"dinv{r}", [128, HALF], fdt, kind="ExternalInput")
              for r in range(3)]
    idx_d = [nc.dram_tensor(f"idx{r}", [128, iw_cols[r]], mybir.dt.int16,
                            kind="ExternalInput") for r in range(3)]
    dstloc_d = [nc.dram_tensor(f"dstloc{r}", [128, dl_cols[r]], bdt,
                               kind="ExternalInput") for r in range(3)]
    out_d = nc.dram_tensor("out", [C_OUT, P_NODES], fdt, kind="ExternalOutput")

    bounces, tables = [], []
    for t in range(6):
        bounces.append(nc.dram_tensor(f"bounce{t}", [n_local, 128], bdt))
        tables.append(nc.dram_tensor(f"table{t}", [N_CORES * n_local, 128], bdt,
                                     addr_space="Shared"))
    rg = [list(range(N_CORES))]

    Ident = mybir.ActivationFunctionType.Identity
    Cp = mybir.ActivationFunctionType.Copy

    def wpart(w):
        return 64 * (w // PK), (w % PK) * 128  # (partition base, column base)

    # window groups of GW, not crossing the packed-half boundary
    groups = []
    for half_start in (0, PK):
        w = half_start
        hi = min(half_start + PK, n_win_real)
        while w < hi:
            gn = min(GW, hi - w)
            groups.append((w, gn))
            w += gn

    def gcols_of(w0, gn):
        return sum(min(WIN, n_local - w * WIN) for w in range(w0, w0 + gn))

    from contextlib import ExitStack
    with tile.TileContext(nc) as tc, ExitStack() as ctx:
        consts = ctx.enter_context(tc.tile_pool(name="consts", bufs=1))
        resid = ctx.enter_context(tc.tile_pool(name="resid", bufs=1))
        hpool = ctx.enter_context(tc.tile_pool(name="hpool", bufs=2))
        f1pool = ctx.enter_context(tc.tile_pool(name="f1pool", bufs=1))
        dinvp = ctx.enter_context(tc.tile_pool(name="dinvp", bufs=2))
        msgp = ctx.enter_context(tc.tile_pool(name="msgp", bufs=6))
        selp = ctx.enter_context(tc.tile_pool(name="selp", bufs=2))
        wtile = ctx.enter_context(tc.tile_pool(name="wtile", bufs=3))
        trp = ctx.enter_context(tc.tile_pool(name="trp", bufs=3))
        psum_seg = ctx.enter_context(
            tc.tile_pool(name="psum_seg", bufs=4, space="PSUM"))
        psum_aux = ctx.enter_context(
            tc.tile_pool(name="psum_aux", bufs=2, space="PSUM"))
        psum_tr = ctx.enter_context(
            tc.tile_pool(name="psum_tr", bufs=2, space="PSUM"))
        idxp = ctx.enter_context(tc.tile_pool(name="idxp", bufs=2))
        idxcp = ctx.enter_context(tc.tile_pool(name="idxcp", bufs=4))

        cw = {}
        for nm in wts:
            dt = fdt if nm.startswith("b") else bdt
            cw[nm] = consts.tile(list(wts[nm].shape), dt, tag=nm, name=f"cw_{nm}")
            nc.sync.dma_start(out=cw[nm][:], in_=wts[nm][:])
        iota_t = cw["iota"]

        h_all = resid.tile([128, HALF], fdt)
        nc.vector.memset(h_all[:], 0.0)

        # ---- initial 2-layer MLP -> h (packed, bf16)
        h_cur = hpool.tile([128, HALF], bdt, tag="h")
        for (w0, gn) in groups:
            pb, cb = wpart(w0)
            gc = gcols_of(w0, gn)
            infw = wtile.tile([IN_F, GW * WIN], bdt, tag="infw",
                              name=f"infw_{w0}")
            nc.sync.dma_start(out=infw[:, :gc],
                              in_=in_featT[:, w0 * WIN:w0 * WIN + gc])
            ps = psum_aux.tile([128, GW * WIN], fdt, tag="aux", name=f"mlpa_{w0}")
            nc.tensor.matmul(out=ps[pb:pb + 64, :gc], lhsT=cw["W1T"][:],
                             rhs=infw[:, :gc], start=True, stop=True,
                             tile_position=(0, pb))
            y1 = wtile.tile([128, GW * WIN], bdt, tag="y1", name=f"y1_{w0}")
            nc.scalar.activation(y1[pb:pb + 64, :gc], ps[pb:pb + 64, :gc], Ident,
                                 bias=cw["b1c"][pb:pb + 64, :])
            h0w = wtile.tile([128, GW * WIN], bdt, tag="h0w", name=f"h0w_{w0}")
            nc.vector.scalar_tensor_tensor(
                out=h0w[pb:pb + 64, :gc], in0=y1[pb:pb + 64, :gc], scalar=0.01,
                in1=y1[pb:pb + 64, :gc],
                op0=mybir.AluOpType.mult, op1=mybir.AluOpType.max)
            ps2 = psum_aux.tile([128, GW * WIN], fdt, tag="aux", name=f"mlpb_{w0}")
            nc.tensor.matmul(out=ps2[pb:pb + 64, :gc],
                             lhsT=cw["W2T"][pb:pb + 64, :],
                             rhs=h0w[pb:pb + 64, :gc], start=True, stop=True,
                             tile_position=(pb, pb))
            y2 = wtile.tile([128, GW * WIN], bdt, tag="y1", name=f"y2_{w0}")
            nc.scalar.activation(y2[pb:pb + 64, :gc], ps2[pb:pb + 64, :gc], Ident,
                                 bias=cw["b2c"][pb:pb + 64, :])
            nc.vector.scalar_tensor_tensor(
                out=h_cur[pb:pb + 64, cb:cb + gc], in0=y2[pb:pb + 64, :gc],
                scalar=0.01, in1=y2[pb:pb + 64, :gc],
                op0=mybir.AluOpType.mult, op1=mybir.AluOpType.max)

        def build_table(src_tile, dinv_t, tbl_idx):
            G = 8
            stage = None
            g0 = 0
            for (w0, gn) in groups:
                pb, cb = wpart(w0)
                gc = gcols_of(w0, gn)
                scl = wtile.tile([128, GW * WIN], bdt, tag="tblscl",
                                 name=f"scl_{w0}")
                nc.vector.tensor_tensor(out=scl[pb:pb + 64, :gc],
                                        in0=src_tile[pb:pb + 64, cb:cb + gc],
                                        in1=dinv_t[pb:pb + 64, cb:cb + gc],
                                        op=mybir.AluOpType.mult)
                for w in range(w0, w0 + gn):
                    wn = min(WIN, n_local - w * WIN)
                    off = (w - w0) * WIN
                    pt = psum_tr.tile([128, H], bdt, tag="tr", name=f"pt_{w}")
                    nc.tensor.transpose(out=pt[:wn, :],
                                        in_=scl[pb:pb + 64, off:off + wn],
                                        identity=cw["identT"][pb:pb + 64, :])
                    if wn == WIN:
                        if stage is None:
                            stage = trp.tile([128, G, H], bdt, tag="trs",
                                             name=f"trs_{tbl_idx}_{w}")
                            g0 = w
                        nc.scalar.activation(stage[:, w - g0, :], pt[:, :], Cp)
                        nxt_wn = min(WIN, n_local - (w + 1) * WIN)
                        if (w - g0 == G - 1 or w == n_win_real - 1
                                or nxt_wn < WIN):
                            gcnt = w - g0 + 1
                            nc.sync.dma_start(
                                out=bounces[tbl_idx][g0 * WIN:(g0 + gcnt) * WIN,
                                                     0:H]
                                    .rearrange("(g p) h -> p g h", p=128),
                                in_=stage[:, 0:gcnt, :])
                            stage = None
                    else:
                        st = trp.tile([128, H], bdt, tag="trs_r",
                                      name=f"trsr_{w}")
                        nc.scalar.activation(st[:wn, :], pt[:wn, :], Cp)
                        nc.sync.dma_start(
                            out=bounces[tbl_idx][w * WIN:w * WIN + wn, 0:H],
                            in_=st[:wn, :])
            nc.gpsimd.collective_compute(
                "AllGather", mybir.AluOpType.bypass, replica_groups=rg,
                ins=[bounces[tbl_idx][:].opt()], outs=[tables[tbl_idx][:].opt()])

        nreg_cache = {}

        def propagate(plan, idx_dram, dl_t, tbl, epilogue):
            built = {}
            msg_tiles = {}
            emitted = [0]
            call_order = sorted(range(len(plan.calls)),
                                key=lambda c: (plan.call_first_window[c], c))
            iw_off = {}
            o = 0
            for cid, (_so, n, _sh) in enumerate(plan.calls):
                iw_off[cid] = o
                o += n // 16

            def emit_calls(up_to_w):
                while emitted[0] < len(call_order):
                    cid = call_order[emitted[0]]
                    if plan.call_first_window[cid] > up_to_w:
                        break
                    so, n, sh = plan.calls[cid]
                    nch = n // 128
                    mt = msgp.tile([128, nch, H], bdt, tag="msg", name=f"msg_{cid}")
                    it = idxcp.tile([128, CALL_MAX // 16], mybir.dt.int16,
                                    tag="idxc", name=f"idxc_{cid}")
                    nc.sync.dma_start(
                        out=it[:, 0:n // 16],
                        in_=idx_dram[:, iw_off[cid]:iw_off[cid] + n // 16])
                    if n not in nreg_cache:
                        nreg_cache[n] = nc.gpsimd.to_reg(n)
                    hi = min((sh + 1) * SHARD_ROWS, N_CORES * n_local)
                    _dma_gather(
                        nc.gpsimd, out_ap=mt[:],
                        in_ap=tbl[sh * SHARD_ROWS:hi, 0:H],
                        idxs_ap=it[:, 0:n // 16],
                        num_idxs=n, num_idxs_reg=nreg_cache[n],
                        elem_size=H, elem_step=128)
                    msg_tiles[cid] = mt
                    emitted[0] += 1

            for (w0, gn) in groups:
                emit_calls(w0 + gn - 1)
                pb, _cb = wpart(w0)
                gc = gcols_of(w0, gn)
                ps = psum_seg.tile([128, GW * WIN], fdt, tag="seg",
                                   name=f"seg_{w0}")
                for w in range(w0, w0 + gn):
                    off = (w - w0) * WIN
                    gids = plan.win_chunks[w]
                    if not gids:
                        wn = min(WIN, n_local - w * WIN)
                        nc.vector.memset(ps[pb:pb + 64, off:off + wn], 0.0)
                    for k, gid in enumerate(gids):
                        ip = int(plan.gid2ipos[gid])
                        bi = ip // SEL_BATCH
                        if bi not in built:
                            i0 = bi * SEL_BATCH
                            nbi = min(SEL_BATCH, plan.n_chunks - i0)
                            st = selp.tile([128, SEL_BATCH * 128], bdt,
                                           tag="sel", name=f"sel_{bi}")
                            nc.vector.tensor_tensor(
                                out=st[:, 0:nbi * 128],
                                in0=dl_t[:, i0:i0 + nbi, None].to_broadcast(
                                    [128, nbi, 128]),
                                in1=iota_t[:, None, :].to_broadcast(
                                    [128, nbi, 128]),
                                op=mybir.AluOpType.is_equal)
                            built[bi] = st
                        st = built[bi]
                        cid, slot = plan.chunk2call[gid]
                        nc.tensor.matmul(
                            out=ps[pb:pb + 64, off:off + WIN],
                            lhsT=msg_tiles[cid][:, slot, :],
                            rhs=st[:, (ip - bi * SEL_BATCH) * 128:
                                   (ip - bi * SEL_BATCH) * 128 + 128],
                            start=(k == 0), stop=(k == len(gids) - 1),
                            tile_position=(0, pb), skip_group_check=True)
                epilogue(w0, gn, gc, ps)

        table_i = 0
        dinv_t = dinvp.tile([128, HALF], fdt, tag="dinv")
        nc.sync.dma_start(out=dinv_t[:], in_=dinv_d[0][:])
        for r in range(3):
            dl_t = idxp.tile([128, dl_cols[r]], bdt, tag="dl", name=f"dl_{r}")
            nc.sync.dma_start(out=dl_t[:], in_=dstloc_d[r][:])

            build_table(h_cur, dinv_t, table_i)

            f1 = f1pool.tile([128, HALF], bdt, tag="f1", name=f"f1_{r}")

            def epi1(w0, gn, gc, ps, f1=f1, dinv_t=dinv_t, h_cur=h_cur):
                pb, cb = wpart(w0)
                tmp = wtile.tile([128, GW * WIN], fdt, tag="scaled",
                                 name=f"ta_{w0}")
                nc.vector.tensor_tensor(out=tmp[pb:pb + 64, :gc],
                                        in0=ps[pb:pb + 64, :gc],
                                        in1=dinv_t[pb:pb + 64, cb:cb + gc],
                                        op=mybir.AluOpType.mult)
                nc.vector.tensor_tensor(out=f1[pb:pb + 64, cb:cb + gc],
                                        in0=h_cur[pb:pb + 64, cb:cb + gc],
                                        in1=tmp[pb:pb + 64, :gc],
                                        op=mybir.AluOpType.subtract)

            propagate(plans[r], idx_d[r], dl_t, tables[table_i], epi1)
            build_table(f1, dinv_t, table_i + 1)

            h_new = hpool.tile([128, HALF], bdt, tag="h", name=f"hn_{r}")
            if r < 2:
                dinv_next = dinvp.tile([128, HALF], fdt, tag="dinv",
                                       name=f"dinv_{r + 1}")
                nc.sync.dma_start(out=dinv_next[:], in_=dinv_d[r + 1][:])

            def epi2(w0, gn, gc, ps, f1=f1, dinv_t=dinv_t, h_cur=h_cur,
                     h_new=h_new):
                pb, cb = wpart(w0)
                tmp = wtile.tile([128, GW * WIN], fdt, tag="scaled",
                                 name=f"tb_{w0}")
                nc.vector.tensor_tensor(out=tmp[pb:pb + 64, :gc],
                                        in0=ps[pb:pb + 64, :gc],
                                        in1=dinv_t[pb:pb + 64, cb:cb + gc],
                                        op=mybir.AluOpType.mult)
                f2w = wtile.tile([128, GW * WIN], bdt, tag="f2w",
                                 name=f"f2_{w0}")
                nc.vector.tensor_tensor(out=f2w[pb:pb + 64, :gc],
                                        in0=f1[pb:pb + 64, cb:cb + gc],
                                        in1=tmp[pb:pb + 64, :gc],
                                        op=mybir.AluOpType.subtract)
                ps3 = psum_aux.tile([128, GW * WIN], fdt, tag="aux",
                                    name=f"w3_{w0}")
                nc.tensor.matmul(out=ps3[pb:pb + 64, :gc],
                                 lhsT=cw["M0T"][pb:pb + 64, :],
                                 rhs=h_cur[pb:pb + 64, cb:cb + gc],
                                 start=True, stop=False, tile_position=(pb, pb))
                nc.tensor.matmul(out=ps3[pb:pb + 64, :gc],
                                 lhsT=cw["M1T"][pb:pb + 64, :],
                                 rhs=f1[pb:pb + 64, cb:cb + gc],
                                 start=False, stop=False, tile_position=(pb, pb))
                nc.tensor.matmul(out=ps3[pb:pb + 64, :gc],
                                 lhsT=cw["M2T"][pb:pb + 64, :],
                                 rhs=f2w[pb:pb + 64, :gc],
                                 start=False, stop=True, tile_position=(pb, pb))
                nc.scalar.activation(h_new[pb:pb + 64, cb:cb + gc],
                                     ps3[pb:pb + 64, :gc], Ident,
                                     bias=cw["b3c"][pb:pb + 64, :])
                nc.vector.tensor_tensor(out=h_all[pb:pb + 64, cb:cb + gc],
                                        in0=h_all[pb:pb + 64, cb:cb + gc],
                                        in1=h_new[pb:pb + 64, cb:cb + gc],
                                        op=mybir.AluOpType.add)

            propagate(plans[r], idx_d[r], dl_t, tables[table_i + 1], epi2)
            table_i += 2
            h_cur = h_new
            if r < 2:
                dinv_t = dinv_next

        # ---- final head
        for (w0, gn) in groups:
            pb, cb = wpart(w0)
            gc = gcols_of(w0, gn)
            lw = wtile.tile([128, GW * WIN], bdt, tag="lrelu", name=f"lr_{w0}")
            nc.vector.scalar_tensor_tensor(
                out=lw[pb:pb + 64, :gc], in0=h_all[pb:pb + 64, cb:cb + gc],
                scalar=0.01, in1=h_all[pb:pb + 64, cb:cb + gc],
                op0=mybir.AluOpType.mult, op1=mybir.AluOpType.max)
            ps = psum_aux.tile([128, GW * WIN], fdt, tag="aux", name=f"hd_{w0}")
            nc.tensor.matmul(out=ps[pb:pb + C_OUT, :gc],
                             lhsT=cw["W4T"][pb:pb + 64, :],
                             rhs=lw[pb:pb + 64, :gc],
                             start=True, stop=True, tile_position=(pb, pb))
            ow = trp.tile([128, GW * WIN], fdt, tag="ow", name=f"ow_{w0}")
            nc.scalar.activation(ow[pb:pb + C_OUT, :gc], ps[pb:pb + C_OUT, :gc],
                                 Ident, bias=cw["b4c"][pb:pb + C_OUT, :])
            nc.sync.dma_start(out=out_d[0:C_OUT, w0 * WIN:w0 * WIN + gc],
                              in_=ow[pb:pb + C_OUT, :gc])

    _insert_library_loads(nc)
    _fix_sync_waits(nc)
    return nc


# ---------------------------------------------------------------- kernel


def prepare(inputs):
    in_feat = np.asarray(inputs["in_feat"], np.float32)
    N = in_feat.shape[0]
    n_local = N // N_CORES
    P_NODES = ((n_local + 255) // 256) * 256
    HALF = P_NODES // 2
    W1 = np.asarray(inputs["W1"], np.float32)
    b1 = np.asarray(inputs["b1"], np.float32)
    W2 = np.asarray(inputs["W2"], np.float32)
    b2 = np.asarray(inputs["b2"], np.float32)
    W3 = np.asarray(inputs["W3"], np.float32)
    b3 = np.asarray(inputs["b3"], np.float32)
    W4 = np.asarray(inputs["W4"], np.float32)
    b4 = np.asarray(inputs["b4"], np.float32)
    srcs = [np.asarray(inputs[f"src{r}"]).astype(np.int64) for r in range(3)]
    dsts = [np.asarray(inputs[f"dst{r}"]).astype(np.int64) for r in range(3)]

    W3a, W3b, W3c = W3[:, 0:H], W3[:, H:2 * H], W3[:, 2 * H:3 * H]
    M = [THETAS[0, k] * W3a + THETAS[1, k] * W3b + THETAS[2, k] * W3c
         for k in range(3)]

    dinvs = []
    for r in range(3):
        deg = np.bincount(dsts[r], minlength=N).astype(np.float32)
        dinvs.append(np.maximum(deg, 1.0) ** -0.5)

    plans = [_plan_relation(srcs[r], dsts[r], N, n_local) for r in range(3)]
    nc = build_nc(plans, n_local)

    def dup(a):
        return np.ascontiguousarray(np.concatenate([a, a], axis=0))

    def dupcol(b):
        col = np.zeros((128, 1), np.float32)
        col[0:len(b), 0] = b
        col[64:64 + len(b), 0] = b
        return col

    iota = np.tile(np.arange(128, dtype=np.float32)[None, :], (128, 1)).astype(BF16)
    identT = dup(np.eye(H, dtype=np.float32)).astype(BF16)
    in_featT = in_feat.T.copy()

    def pack(a):
        if a.ndim == 1:
            a = np.tile(a[None, :], (H, 1))
        return np.ascontiguousarray(
            np.concatenate([a[:, :HALF], a[:, HALF:]], axis=0))

    in_maps = []
    for c in range(N_CORES):
        m = {
            "in_featT": np.ascontiguousarray(
                np.pad(in_featT[:, c * n_local:(c + 1) * n_local],
                       ((0, 0), (0, P_NODES - n_local)))).astype(BF16),
            "W1T": W1.T.copy().astype(BF16), "W2T": dup(W2.T).astype(BF16),
            "M0T": dup(M[0].T).astype(BF16), "M1T": dup(M[1].T).astype(BF16),
            "M2T": dup(M[2].T).astype(BF16), "W4T": dup(W4.T).astype(BF16),
            "b1c": dupcol(b1), "b2c": dupcol(b2), "b3c": dupcol(b3),
            "b4c": dupcol(b4),
            "iota": iota, "identT": identT,
        }
        for r in range(3):
            dl = np.pad(dinvs[r][c * n_local:(c + 1) * n_local],
                        (0, P_NODES - n_local))
            m[f"dinv{r}"] = pack(dl)
            m[f"idx{r}"] = np.ascontiguousarray(plans[r].idx_wrapped[c])
            m[f"dstloc{r}"] = np.ascontiguousarray(plans[r].dstloc[c])
        in_maps.append(m)
    return nc, in_maps, n_local


def kernel(**inputs):
    global LAST_BUILD
    nc, in_maps, n_local = prepare(inputs)
    _lower_library_reloads(nc)
    LAST_BUILD = (nc, in_maps)
    from concourse.bass_utils import run_bass_kernel_spmd
    res = run_bass_kernel_spmd(nc, in_maps, core_ids=list(range(N_CORES)))
    outs = [res.results[c]["out"][:, :n_local] for c in range(N_CORES)]
    return np.ascontiguousarray(np.concatenate(outs, axis=1).T)
